# revision 13
# baseline (speedup 1.0000x reference)
"""GPT forward (L=12, D=1024, H=16, B=2, T=1024, V=32000) on 8 trn2 NeuronCores.

Sharding: sequence-parallel. Core c owns batch c//4, token chunk c%4 (256
tokens = two 128-halves A/B, global 128-chunks 2j and 2j+1).

Per layer, software-pipelined so the K/V AllGathers hide under compute:
  attention(i)+proj(i) -> LN2_A+MLP_A -> LN1_A(i+1)+K_A,V_A+ship+AG_A ->
  LN2_B+MLP_B -> LN1_B(i+1)+K_B,V_B+ship+AG_B -> Q(i+1)
Each AG moves 2MB (one half's K^T,V from the 4 cores of the batch group);
AG_A is covered by MLP_B+KV_B+Q, AG_B by Q plus the even-kv scores of the
next attention (scores are grouped by kv parity: even chunks come from AG_A,
odd from AG_B).

Attention: transposed-score layout, per head 8 score matmuls (one per kv
128-chunk) packed into two [128,1024] PSUM quads by kv parity, exp on
scalar, mask-mul on vector, AV matmuls delayed one head (denominator via
ones-column in vext). proj (+residual) -> LN2 -> fc1 -> exact GELU -> fc2.

Final: LN -> per-half AllGather of hidden states (all 8 cores, Shared) ->
vocab-sharded logits matmul, half-0 tokens first so AG_f1 overlaps.
Everything bf16 into the PE with fp32 PSUM accumulation; residuals fp32.
"""
import sys
import numpy as np

sys.path.insert(0, '/opt/trn_rl_repo')
import ml_dtypes

BF = ml_dtypes.bfloat16
L, D, H, V, B, T = 12, 1024, 16, 32000, 2, 1024
DH = D // H          # 64
EPS = 1e-5
N_CORES = 8
CHUNK = 256          # tokens per core
HALF = 128           # tokens per pipeline half
VS = V // N_CORES    # 4000 vocab cols per core
KT = 8               # kv chunks of 128 per batch
FT = D // 128        # 8 feature tiles


def host_prep(inputs):
    inputs = {k: np.asarray(v) for k, v in inputs.items()}
    for name in ['ln1_b', 'ln2_b', 'b_qkv', 'b_proj', 'b_fc1', 'b_fc2', 'lnf_b']:
        assert not np.any(inputs[name]), f"{name} nonzero — bias folding unsupported"
    x0 = inputs['wte'][inputs['tokens']] + inputs['wpe'][None, :, :]   # [B,T,D] f32
    w_qkv = inputs['w_qkv'] * inputs['ln1_w'][:, :, None]
    w_fc1 = inputs['w_fc1'] * inputs['ln2_w'][:, :, None]
    w_out = inputs['w_out'] * inputs['lnf_w'][:, None]
    return {
        'x0': np.ascontiguousarray(x0, np.float32),
        'w_qkv': np.ascontiguousarray(w_qkv.astype(BF)),
        'w_proj': np.ascontiguousarray(inputs['w_proj'].astype(BF)),
        'w_fc1': np.ascontiguousarray(w_fc1.astype(BF)),
        'w_fc2': np.ascontiguousarray(inputs['w_fc2'].astype(BF)),
        'w_out': np.ascontiguousarray(w_out.astype(BF)),
    }


def make_masks(j):
    """Causal masks, transposed-score layout, kv chunks packed by PARITY:
    quad p (p=0 even kv chunks, p=1 odd), block tp in 0..3 holds global kv
    chunk t = 2*tp + p.  mask[p][row, 256*tp + q] = (128*t + row) <= (256*j + q)."""
    out = np.zeros((2, 128, 4 * CHUNK), BF)
    for p in range(2):
        for tp in range(4):
            t = 2 * tp + p
            kv = 128 * t + np.arange(128)[:, None]
            qp = 256 * j + np.arange(CHUNK)[None, :]
            out[p][:, 256 * tp:256 * (tp + 1)] = (kv <= qp).astype(BF)
    return out


def build_program(n_layers=L, bcast_mode='mm'):
    import concourse.bass as bass
    import concourse.mybir as mybir
    import concourse.tile as tile
    from concourse import bacc
    from concourse.masks import make_identity
    from contextlib import ExitStack

    f32 = mybir.dt.float32
    bf16 = mybir.dt.bfloat16
    AF = mybir.ActivationFunctionType

    nc = bacc.Bacc('TRN2', target_bir_lowering=False, debug=False, num_devices=N_CORES)

    x0_in = nc.dram_tensor("x0", [CHUNK, D], f32, kind="ExternalInput")
    wq_in = nc.dram_tensor("wq", [n_layers, D, 3 * D], bf16, kind="ExternalInput")
    wp_in = nc.dram_tensor("wp", [n_layers, D, D], bf16, kind="ExternalInput")
    w1_in = nc.dram_tensor("w1", [n_layers, D, D], bf16, kind="ExternalInput")
    w2_in = nc.dram_tensor("w2", [n_layers, D, D], bf16, kind="ExternalInput")
    wo_in = nc.dram_tensor("wo", [D, VS], bf16, kind="ExternalInput")
    mk_in = nc.dram_tensor("masks", [2, 128, 4 * CHUNK], bf16, kind="ExternalInput")
    out_ext = nc.dram_tensor("logits", [N_CORES * CHUNK, VS], f32, kind="ExternalOutput")

    # per-layer, per-half collective buffers: K^T [D,128] then V [128,D] flat.
    KVH = 2 * D * HALF                      # elements per rank contribution
    kv_locs = [[nc.dram_tensor(f"kvl_{l}_{m}", [2 * D, HALF], bf16)
                for m in range(2)] for l in range(n_layers)]
    kv_alls = [[nc.dram_tensor(f"kva_{l}_{m}", [4 * 2 * D, HALF], bf16)
                for m in range(2)] for l in range(n_layers)]
    xf_locs = [nc.dram_tensor(f"xfl_{m}", [D, HALF], bf16) for m in range(2)]
    xf_alls = [nc.dram_tensor(f"xfa_{m}", [N_CORES * D, HALF], bf16,
                              addr_space="Shared") for m in range(2)]

    groups_b = [[0, 1, 2, 3], [4, 5, 6, 7]]
    group_all = [list(range(N_CORES))]

    def dram_ap(handle, offset, ap):
        base = handle[:, :]
        return bass.AP(tensor=base.tensor, offset=offset, ap=ap)

    def _patch_tile_name(pool):
        orig = pool.tile
        def tile(shape, dtype, *, tag="", **kw):
            kw.setdefault("name", tag or "t")
            return orig(shape, dtype, tag=tag, **kw)
        pool.tile = tile
        return pool

    with tile.TileContext(nc) as tc, ExitStack() as ctx:
        persist = _patch_tile_name(ctx.enter_context(tc.tile_pool(name="persist", bufs=1)))
        x_t = [persist.tile([128, D], f32, tag=f"x{m}") for m in range(2)]
        for m in range(2):
            nc.sync.dma_start(out=x_t[m], in_=x0_in[128 * m:128 * (m + 1), :])
        ident = persist.tile([128, 128], bf16, tag="ident")
        make_identity(nc, ident)
        eps_t = persist.tile([128, 1], f32, tag="eps")
        nc.vector.memset(eps_t, EPS)
        mask_t = [persist.tile([128, 4 * CHUNK], bf16, tag=f"mask{p}") for p in range(2)]
        for p in range(2):
            nc.sync.dma_start(out=mask_t[p], in_=mk_in[p, :, :])

        ln_pool = _patch_tile_name(ctx.enter_context(tc.tile_pool(name="ln", bufs=1)))
        wqpool = _patch_tile_name(ctx.enter_context(tc.tile_pool(name="wq", bufs=1)))
        wpool = _patch_tile_name(ctx.enter_context(tc.tile_pool(name="wsmall", bufs=1)))
        apool = _patch_tile_name(ctx.enter_context(tc.tile_pool(name="acts", bufs=1)))
        kvpool = _patch_tile_name(ctx.enter_context(tc.tile_pool(name="kv", bufs=1)))
        epool = _patch_tile_name(ctx.enter_context(tc.tile_pool(name="eexp", bufs=1)))
        spool = _patch_tile_name(ctx.enter_context(tc.tile_pool(name="small", bufs=2)))

        ps_s = _patch_tile_name(ctx.enter_context(tc.tile_pool(name="ps_s", bufs=2, space="PSUM")))
        ps_g = _patch_tile_name(ctx.enter_context(tc.tile_pool(name="ps_g", bufs=4, space="PSUM")))

        # persistent weight tiles
        wq_t = [wqpool.tile([128, 3 * D], bf16, tag=f"wq{kk}") for kk in range(FT)]
        wp_t = [wpool.tile([128, D], bf16, tag=f"wp{kk}") for kk in range(FT)]
        w1_t = [wpool.tile([128, D], bf16, tag=f"w1{kk}") for kk in range(FT)]
        w2_t = [wpool.tile([128, D], bf16, tag=f"w2{kk}") for kk in range(FT)]

        # LN outputs (feature-major, both halves side by side)
        xhT = [ln_pool.tile([128, CHUNK], bf16, tag=f"xhT{t}") for t in range(FT)]
        hT = [ln_pool.tile([128, CHUNK], bf16, tag=f"hT{t}") for t in range(FT)]
        qT = [apool.tile([128, CHUNK], bf16, tag=f"qT{t}") for t in range(FT)]
        attnT = [apool.tile([128, CHUNK], bf16, tag=f"aT{t}") for t in range(FT)]
        gT = [apool.tile([128, CHUNK], bf16, tag=f"gT{t}") for t in range(FT)]
        kship = [apool.tile([128, 128], bf16, tag=f"ks{t}") for t in range(FT)]
        vship = apool.tile([128, D], bf16, tag="vs")
        # gathered K^T by kv parity: kall_p[par][ft] [128, 512] (4 chunks)
        kall_p = [[kvpool.tile([128, 4 * 128], bf16, tag=f"ka{par}_{t}")
                   for t in range(FT)] for par in range(2)]
        vext = [kvpool.tile([128, 16 * 65], bf16, tag=f"vext{t}") for t in range(KT)]

        def load_wq(li, what):
            for kk in range(FT):
                r0, r1 = 128 * kk, 128 * (kk + 1)
                if what == 'k':
                    nc.sync.dma_start(out=wq_t[kk][:, D:2 * D], in_=wq_in[li, r0:r1, D:2 * D])
                else:
                    nc.sync.dma_start(out=wq_t[kk][:, 0:D], in_=wq_in[li, r0:r1, 0:D])
                    nc.sync.dma_start(out=wq_t[kk][:, 2 * D:3 * D], in_=wq_in[li, r0:r1, 2 * D:3 * D])

        def load_w(dst, src, li):
            for kk in range(FT):
                nc.sync.dma_start(out=dst[kk], in_=src[li, 128 * kk:128 * (kk + 1), :])

        _ln_ctr = [0]

        def ln_stats(m):
            """Vector part of LN on x_t[m]: returns normalized bf16 tile."""
            stats = spool.tile([128, 2, 6], f32, tag="lnstats")
            nc.vector.bn_stats(out=stats[:, 0, :], in_=x_t[m][:, 0:512])
            nc.vector.bn_stats(out=stats[:, 1, :], in_=x_t[m][:, 512:1024])
            mv = spool.tile([128, 2], f32, tag="lnmv")
            nc.vector.bn_aggr(out=mv, in_=stats)
            rs = spool.tile([128, 1], f32, tag="lnrs")
            nc.scalar.activation(out=rs, in_=mv[:, 1:2], func=AF.Sqrt, bias=eps_t)
            nc.vector.reciprocal(out=rs, in_=rs)
            xh_m = ln_pool.tile([128, D], bf16, tag=f"lnxh{_ln_ctr[0] % 2}")
            _ln_ctr[0] += 1
            nc.vector.tensor_scalar(
                out=xh_m, in0=x_t[m], scalar1=mv[:, 0:1], scalar2=rs,
                op0=mybir.AluOpType.subtract, op1=mybir.AluOpType.mult)
            return xh_m

        def ln_transposes(xh_m, m, dst):
            """PE part of LN: transpose xh_m into dst[t][:, 128m:128m+128]."""
            for t in range(FT):
                ptr = ps_g.tile([128, 128], bf16, tag="g", padded_shape=[128, 512])
                nc.tensor.transpose(ptr, xh_m[:, 128 * t:128 * (t + 1)], ident)
                eng = nc.vector if t % 2 == 0 else nc.scalar
                if eng is nc.scalar:
                    nc.scalar.copy(out=dst[t][:, 128 * m:128 * (m + 1)], in_=ptr)
                else:
                    nc.vector.tensor_copy(out=dst[t][:, 128 * m:128 * (m + 1)], in_=ptr)

        def layernorm_half(m, dst):
            ln_transposes(ln_stats(m), m, dst)

        def kv_half(l, m):
            """K^T,V for half m of layer l from xhT; ship; AllGather."""
            kv_loc, kv_all = kv_locs[l][m], kv_alls[l][m]
            c0 = 128 * m
            for ft in range(FT):
                ps = ps_g.tile([128, 512], f32, tag="g")
                for kk in range(FT):
                    nc.tensor.matmul(ps[:, 0:128], wq_t[kk][:, D + 128 * ft:D + 128 * (ft + 1)],
                                     xhT[kk][:, c0:c0 + 128],
                                     start=(kk == 0), stop=(kk == FT - 1))
                eng = nc.vector if ft % 2 == 0 else nc.scalar
                if eng is nc.scalar:
                    nc.scalar.copy(out=kship[ft], in_=ps[:, 0:128])
                else:
                    nc.vector.tensor_copy(out=kship[ft], in_=ps[:, 0:128])
                nc.sync.dma_start(
                    out=dram_ap(kv_loc, 128 * ft * HALF, [[HALF, 128], [1, HALF]]),
                    in_=kship[ft])
            for n in range(2):
                ps = ps_g.tile([128, 512], f32, tag="g")
                for kk in range(FT):
                    nc.tensor.matmul(
                        ps, xhT[kk][:, c0:c0 + 128],
                        wq_t[kk][:, 2 * D + 512 * n:2 * D + 512 * (n + 1)],
                        start=(kk == 0), stop=(kk == FT - 1))
                nc.vector.tensor_copy(out=vship[:, 512 * n:512 * (n + 1)], in_=ps)
            nc.sync.dma_start(
                out=dram_ap(kv_loc, D * HALF, [[D, 128], [1, D]]),
                in_=vship)
            nc.gpsimd.collective_compute(
                "AllGather", mybir.AluOpType.bypass, replica_groups=groups_b,
                ins=[kv_loc[:, :]], outs=[kv_all[:, :]])
            # pull this parity's gathered K/V as soon as the AG lands
            par = m
            for ft in range(FT):
                nc.sync.dma_start(
                    out=kall_p[par][ft].rearrange("p (r c) -> p r c", r=4),
                    in_=dram_ap(kv_all, 128 * ft * HALF,
                                [[HALF, 128], [KVH, 4], [1, HALF]]))
            for r in range(4):
                t = 2 * r + par
                ve = vext[t].rearrange("p (h c) -> p h c", h=16)
                nc.sync.dma_start(
                    out=ve[:, :, 0:64],
                    in_=dram_ap(kv_all, KVH * r + D * HALF,
                                [[D, 128], [64, 16], [1, 64]]))

        def q_both(l):
            for ft in range(FT):
                ps = ps_g.tile([128, 512], f32, tag="g")
                for kk in range(FT):
                    nc.tensor.matmul(ps[:, 0:CHUNK], wq_t[kk][:, 128 * ft:128 * (ft + 1)],
                                     xhT[kk], start=(kk == 0), stop=(kk == FT - 1))
                eng = nc.vector if ft % 2 == 0 else nc.scalar
                if eng is nc.scalar:
                    nc.scalar.copy(out=qT[ft], in_=ps[:, 0:CHUNK])
                else:
                    eng.tensor_copy(out=qT[ft], in_=ps[:, 0:CHUNK])

        def attention(l):
            def emit_scores(h, par):
                ft, ro = h // 2, 64 * (h % 2)
                s_ps = ps_s.tile([128, 4 * CHUNK], f32, tag="squad")
                for r in range(4):
                    nc.tensor.matmul(
                        s_ps[:, 256 * r:256 * (r + 1)],
                        kall_p[par][ft][ro:ro + 64, 128 * r:128 * (r + 1)],
                        qT[ft][ro:ro + 64, :], start=True, stop=True)
                em_q = epool.tile([128, 4 * CHUNK], bf16, tag=f"emq{par}",
                                  bufs=(3 if par == 0 else 2))
                nc.scalar.activation(out=em_q, in_=s_ps, func=AF.Exp, scale=0.125)
                nc.vector.tensor_mul(out=em_q, in0=em_q, in1=mask_t[par])
                return em_q

            def emit_av(h, em_pair):
                att_ps = ps_g.tile([65, CHUNK], f32, tag="g")
                for t in range(KT):
                    par, r = t % 2, t // 2
                    nc.tensor.matmul(att_ps, vext[t][:, 65 * h:65 * h + 65],
                                     em_pair[par][:, 256 * r:256 * (r + 1)],
                                     start=(t == 0), stop=(t == KT - 1))
                return att_ps

            def emit_epilogue(h, att_ps):
                ft, ro = h // 2, 64 * (h % 2)
                d_sb = spool.tile([1, CHUNK], f32, tag="denom", bufs=3)
                nc.vector.tensor_copy(out=d_sb, in_=att_ps[64:65, :])
                r_sb = spool.tile([1, CHUNK], f32, tag="recip", bufs=3)
                nc.vector.reciprocal_approx_fast(out=r_sb, in_=d_sb)
                rb_sb = spool.tile([64, CHUNK], f32, tag="rbsb", bufs=3)
                nc.gpsimd.partition_broadcast(rb_sb, r_sb, channels=64)
                nc.vector.tensor_mul(out=attnT[ft][ro:ro + 64, :],
                                     in0=att_ps[0:64, :], in1=rb_sb)

            DQ = 2   # odd-parity scores delayed DQ heads so AG_B has cover
            em_e, em_o, av_ps = {}, {}, {}
            for step in range(H + DQ + 2):
                if step < H:
                    em_e[step] = emit_scores(step, 0)
                ho = step - DQ
                if 0 <= ho < H:
                    em_o[ho] = emit_scores(ho, 1)
                ha = step - DQ - 1
                if 0 <= ha < H:
                    av_ps[ha] = emit_av(ha, [em_e.pop(ha), em_o.pop(ha)])
                he = step - DQ - 2
                if 0 <= he < H:
                    emit_epilogue(he, av_ps.pop(he))

        def proj_half(m):
            for n in range(2):
                ps = ps_g.tile([128, 512], f32, tag="g")
                for kk in range(FT):
                    nc.tensor.matmul(
                        ps, attnT[kk][:, 128 * m:128 * (m + 1)],
                        wp_t[kk][:, 512 * n:512 * (n + 1)],
                        start=(kk == 0), stop=(kk == FT - 1))
                nc.vector.tensor_add(
                    out=x_t[m][:, 512 * n:512 * (n + 1)],
                    in0=x_t[m][:, 512 * n:512 * (n + 1)], in1=ps)

        def fc1_half(m):
            c0 = 128 * m
            for f in range(FT):
                ps = ps_g.tile([128, 512], f32, tag="g")
                for kk in range(FT):
                    nc.tensor.matmul(ps[:, 0:128], w1_t[kk][:, 128 * f:128 * (f + 1)],
                                     hT[kk][:, c0:c0 + 128],
                                     start=(kk == 0), stop=(kk == FT - 1))
                nc.scalar.activation(out=gT[f][:, c0:c0 + 128], in_=ps[:, 0:128],
                                     func=AF.Gelu)

        def fc2_half(m):
            c0 = 128 * m
            for n in range(2):
                ps = ps_g.tile([128, 512], f32, tag="g")
                for kk in range(FT):
                    nc.tensor.matmul(
                        ps, gT[kk][:, c0:c0 + 128],
                        w2_t[kk][:, 512 * n:512 * (n + 1)],
                        start=(kk == 0), stop=(kk == FT - 1))
                nc.vector.tensor_add(
                    out=x_t[m][:, 512 * n:512 * (n + 1)],
                    in0=x_t[m][:, 512 * n:512 * (n + 1)], in1=ps)

        def lnf_ship(m, xfT):
            for t in range(FT):
                nc.sync.dma_start(
                    out=dram_ap(xf_locs[m], 128 * t * HALF, [[HALF, 128], [1, HALF]]),
                    in_=xfT[t][:, 128 * m:128 * (m + 1)])
            nc.gpsimd.collective_compute(
                "AllGather", mybir.AluOpType.bypass, replica_groups=group_all,
                ins=[xf_locs[m][:, :]], outs=[xf_alls[m][:, :]])

        def forward():
            # prologue: layer 0 weights, LN1 both halves, KV both, Q
            load_wq(0, 'k')
            load_wq(0, 'qv')
            load_w(wp_t, wp_in, 0)
            load_w(w1_t, w1_in, 0)
            load_w(w2_t, w2_in, 0)
            for t in range(KT):   # ones column for AV denominators, written once
                ve = vext[t].rearrange("p (h c) -> p h c", h=16)
                nc.gpsimd.memset(ve[:, :, 64:65], 1.0)
            layernorm_half(0, xhT)
            kv_half(0, 0)
            layernorm_half(1, xhT)
            kv_half(0, 1)
            q_both(0)

            xfT = hT  # reuse LN2 tiles for final LN output
            for l in range(n_layers):
                last = l + 1 >= n_layers
                attention(l)
                if not last:
                    load_wq(l + 1, 'k')
                    load_wq(l + 1, 'qv')
                proj_half(0)
                xh2a = ln_stats(0)          # LN2_A vector chain under proj_B
                proj_half(1)
                ln_transposes(xh2a, 0, hT)
                xh2b = ln_stats(1)          # LN2_B under fc1_A
                fc1_half(0)
                fc2_half(0)                 # -> x_A of next layer
                xh1a = ln_stats(0)          # LN1_A(l+1) / lnf_A under fc1_B
                ln_transposes(xh2b, 1, hT)
                fc1_half(1)
                fc2_half(1)                 # -> x_B
                if not last:
                    ln_transposes(xh1a, 0, xhT)
                    kv_half(l + 1, 0)       # + AG_A + even-parity loads
                    xh1b = ln_stats(1)      # LN1_B under KV_A
                    load_w(wp_t, wp_in, l + 1)
                    load_w(w1_t, w1_in, l + 1)
                    load_w(w2_t, w2_in, l + 1)
                    ln_transposes(xh1b, 1, xhT)
                    kv_half(l + 1, 1)       # + AG_B + odd-parity loads
                    q_both(l + 1)
                else:
                    ln_transposes(xh1a, 0, xfT)
                    lnf_ship(0, xfT)
                    xh1b = ln_stats(1)
                    ln_transposes(xh1b, 1, xfT)
                    lnf_ship(1, xfT)

            # ---- logits: xall cols = [half0 ranks 0..7 | half1 ranks 0..7] ----
            xall = [wqpool.tile([128, N_CORES * HALF * 2], bf16, tag=f"wq{t}")
                    for t in range(FT)]
            for m in range(2):
                for t in range(FT):
                    nc.sync.dma_start(
                        out=xall[t].rearrange("p (r c) -> p r c", r=16)[:, 8 * m:8 * (m + 1), :],
                        in_=dram_ap(xf_alls[m], 128 * t * HALF,
                                    [[HALF, 128], [D * HALF, N_CORES], [1, HALF]]))
            NCH = [512] * 7 + [VS - 512 * 7]
            for n in range(8):
                n0 = 512 * n
                won = [wpool.tile([128, 512], bf16, tag=f"won{kk}", bufs=2) for kk in range(FT)]
                for kk in range(FT):
                    nc.sync.dma_start(out=won[kk][:, :NCH[n]],
                                      in_=wo_in[128 * kk:128 * (kk + 1), n0:n0 + NCH[n]])
                for mm in range(16):
                    # mm -> (half, rank): halves in order so AG_f1 overlaps
                    m, r = mm // 8, mm % 8
                    # core r holds batch r//4, chunk r%4; half m = tokens
                    # 256*(r%4)+128*m .. +128 of that batch
                    row0 = 1024 * (r // 4) + 256 * (r % 4) + 128 * m
                    ps = ps_g.tile([128, 512], f32, tag="g")
                    for kk in range(FT):
                        nc.tensor.matmul(
                            ps[:, :NCH[n]], xall[kk][:, 128 * mm:128 * (mm + 1)],
                            won[kk][:, :NCH[n]],
                            start=(kk == 0), stop=(kk == FT - 1))
                    lg = ln_pool.tile([128, 512], f32, tag="lg", bufs=2)
                    eng = nc.vector if mm % 2 == 0 else nc.scalar
                    if eng is nc.scalar:
                        nc.scalar.copy(out=lg[:, :NCH[n]], in_=ps[:, :NCH[n]])
                    else:
                        nc.vector.tensor_copy(out=lg[:, :NCH[n]], in_=ps[:, :NCH[n]])
                    nc.sync.dma_start(
                        out=out_ext[row0:row0 + 128, n0:n0 + NCH[n]],
                        in_=lg[:, :NCH[n]])

        forward()

    nc.compile()
    return nc


_CACHE = {}


def _get_program(n_layers=L, bcast_mode='mm'):
    key = (n_layers, bcast_mode)
    if key not in _CACHE:
        _CACHE[key] = build_program(n_layers, bcast_mode)
    return _CACHE[key]


def build_in_maps(prep, n_layers=L):
    in_maps = []
    for c in range(N_CORES):
        b, j = c // 4, c % 4
        in_maps.append({
            'x0': np.ascontiguousarray(prep['x0'][b, 256 * j:256 * (j + 1), :]),
            'wq': prep['w_qkv'][:n_layers],
            'wp': prep['w_proj'][:n_layers],
            'w1': prep['w_fc1'][:n_layers],
            'w2': prep['w_fc2'][:n_layers],
            'wo': np.ascontiguousarray(prep['w_out'][:, VS * c:VS * (c + 1)]),
            'masks': make_masks(j),
        })
    return in_maps


def unshard(res, n_rep=1):
    parts = [res.results[c]['logits'] for c in range(N_CORES)]   # [2048, 4000] each
    full = np.concatenate(parts, axis=1)                          # [2048, 32000]
    return full.reshape(B, T, V)


def run_model(prep, n_layers=L, bcast_mode='mm', **run_kwargs):
    from concourse.bass_utils import run_bass_kernel_spmd
    nc = _get_program(n_layers, bcast_mode)
    in_maps = build_in_maps(prep, n_layers)
    res = run_bass_kernel_spmd(nc, in_maps, core_ids=list(range(N_CORES)), **run_kwargs)
    return unshard(res)


def kernel(**inputs):
    prep = host_prep(inputs)
    return run_model(prep)


# revision 18
# speedup vs baseline: 1.0397x; 1.0397x over previous
"""GPT forward (L=12, D=1024, H=16, B=2, T=1024, V=32000) on 8 trn2 NeuronCores.

Sharding: sequence-parallel. Core c owns batch c//4, token chunk c%4 (256
tokens = two 128-halves A/B, global 128-chunks 2j and 2j+1).

Per layer, software-pipelined so the K/V AllGathers hide under compute:
  attention(i)+proj(i) -> LN2_A+MLP_A -> LN1_A(i+1)+K_A,V_A+ship+AG_A ->
  LN2_B+MLP_B -> LN1_B(i+1)+K_B,V_B+ship+AG_B -> Q(i+1)
Each AG moves 2MB (one half's K^T,V from the 4 cores of the batch group);
AG_A is covered by MLP_B+KV_B+Q, AG_B by Q plus the even-kv scores of the
next attention (scores are grouped by kv parity: even chunks come from AG_A,
odd from AG_B).

Attention: transposed-score layout, per head 8 score matmuls (one per kv
128-chunk) packed into two [128,1024] PSUM quads by kv parity, exp on
scalar, mask-mul on vector, AV matmuls delayed one head (denominator via
ones-column in vext). proj (+residual) -> LN2 -> fc1 -> exact GELU -> fc2.

Final: LN -> per-half AllGather of hidden states (all 8 cores, Shared) ->
vocab-sharded logits matmul, half-0 tokens first so AG_f1 overlaps.
Everything bf16 into the PE with fp32 PSUM accumulation; residuals fp32.
"""
import sys
import numpy as np

sys.path.insert(0, '/opt/trn_rl_repo')
import ml_dtypes

BF = ml_dtypes.bfloat16
L, D, H, V, B, T = 12, 1024, 16, 32000, 2, 1024
DH = D // H          # 64
EPS = 1e-5
N_CORES = 8
CHUNK = 256          # tokens per core
HALF = 128           # tokens per pipeline half
VS = V // N_CORES    # 4000 vocab cols per core
KT = 8               # kv chunks of 128 per batch
FT = D // 128        # 8 feature tiles


def host_prep(inputs):
    inputs = {k: np.asarray(v) for k, v in inputs.items()}
    for name in ['ln1_b', 'ln2_b', 'b_qkv', 'b_proj', 'b_fc1', 'b_fc2', 'lnf_b']:
        assert not np.any(inputs[name]), f"{name} nonzero — bias folding unsupported"
    x0 = inputs['wte'][inputs['tokens']] + inputs['wpe'][None, :, :]   # [B,T,D] f32
    w_qkv = inputs['w_qkv'] * inputs['ln1_w'][:, :, None]
    w_fc1 = inputs['w_fc1'] * inputs['ln2_w'][:, :, None]
    w_out = inputs['w_out'] * inputs['lnf_w'][:, None]
    return {
        'x0': np.ascontiguousarray(x0, np.float32),
        'w_qkv': np.ascontiguousarray(w_qkv.astype(BF)),
        'w_proj': np.ascontiguousarray(inputs['w_proj'].astype(BF)),
        'w_fc1': np.ascontiguousarray(w_fc1.astype(BF)),
        'w_fc2': np.ascontiguousarray(inputs['w_fc2'].astype(BF)),
        'w_out': np.ascontiguousarray(w_out.astype(BF)),
    }


def make_masks(j):
    """Causal masks, transposed-score layout, kv chunks packed by PARITY:
    quad p (p=0 even kv chunks, p=1 odd), block tp in 0..3 holds global kv
    chunk t = 2*tp + p.  mask[p][row, 256*tp + q] = (128*t + row) <= (256*j + q)."""
    out = np.zeros((2, 128, 4 * CHUNK), BF)
    for p in range(2):
        for tp in range(4):
            t = 2 * tp + p
            kv = 128 * t + np.arange(128)[:, None]
            qp = 256 * j + np.arange(CHUNK)[None, :]
            out[p][:, 256 * tp:256 * (tp + 1)] = (kv <= qp).astype(BF)
    return out


def build_program(n_layers=L, bcast_mode='mm'):
    import concourse.bass as bass
    import concourse.mybir as mybir
    import concourse.tile as tile
    from concourse import bacc
    from concourse.masks import make_identity
    from contextlib import ExitStack

    f32 = mybir.dt.float32
    bf16 = mybir.dt.bfloat16
    AF = mybir.ActivationFunctionType

    nc = bacc.Bacc('TRN2', target_bir_lowering=False, debug=False, num_devices=N_CORES)

    x0_in = nc.dram_tensor("x0", [CHUNK, D], f32, kind="ExternalInput")
    wq_in = nc.dram_tensor("wq", [n_layers, D, 3 * D], bf16, kind="ExternalInput")
    wp_in = nc.dram_tensor("wp", [n_layers, D, D], bf16, kind="ExternalInput")
    w1_in = nc.dram_tensor("w1", [n_layers, D, D], bf16, kind="ExternalInput")
    w2_in = nc.dram_tensor("w2", [n_layers, D, D], bf16, kind="ExternalInput")
    wo_in = nc.dram_tensor("wo", [D, VS], bf16, kind="ExternalInput")
    mk_in = nc.dram_tensor("masks", [2, 128, 4 * CHUNK], bf16, kind="ExternalInput")
    out_ext = nc.dram_tensor("logits", [N_CORES * CHUNK, VS], f32, kind="ExternalOutput")

    # per-layer, per-half collective buffers: K^T [D,128] then V [128,D] flat.
    KVH = 2 * D * HALF                      # elements per rank contribution
    kv_locs = [[nc.dram_tensor(f"kvl_{l}_{m}", [2 * D, HALF], bf16)
                for m in range(2)] for l in range(n_layers)]
    kv_alls = [[nc.dram_tensor(f"kva_{l}_{m}", [4 * 2 * D, HALF], bf16)
                for m in range(2)] for l in range(n_layers)]
    xf_locs = [nc.dram_tensor(f"xfl_{m}", [D, HALF], bf16) for m in range(2)]
    xf_alls = [nc.dram_tensor(f"xfa_{m}", [N_CORES * D, HALF], bf16,
                              addr_space="Shared") for m in range(2)]

    groups_b = [[0, 1, 2, 3], [4, 5, 6, 7]]
    group_all = [list(range(N_CORES))]

    def dram_ap(handle, offset, ap):
        base = handle[:, :]
        return bass.AP(tensor=base.tensor, offset=offset, ap=ap)

    def _patch_tile_name(pool):
        orig = pool.tile
        def tile(shape, dtype, *, tag="", **kw):
            kw.setdefault("name", tag or "t")
            return orig(shape, dtype, tag=tag, **kw)
        pool.tile = tile
        return pool

    with tile.TileContext(nc) as tc, ExitStack() as ctx:
        persist = _patch_tile_name(ctx.enter_context(tc.tile_pool(name="persist", bufs=1)))
        x_t = [persist.tile([128, D], f32, tag=f"x{m}") for m in range(2)]
        for m in range(2):
            nc.sync.dma_start(out=x_t[m], in_=x0_in[128 * m:128 * (m + 1), :])
        ident = persist.tile([128, 128], bf16, tag="ident")
        make_identity(nc, ident)
        eps_t = persist.tile([128, 1], f32, tag="eps")
        nc.vector.memset(eps_t, EPS)
        mask_t = [persist.tile([128, 4 * CHUNK], bf16, tag=f"mask{p}") for p in range(2)]
        for p in range(2):
            nc.sync.dma_start(out=mask_t[p], in_=mk_in[p, :, :])

        ln_pool = _patch_tile_name(ctx.enter_context(tc.tile_pool(name="ln", bufs=1)))
        wqpool = _patch_tile_name(ctx.enter_context(tc.tile_pool(name="wq", bufs=1)))
        wpool = _patch_tile_name(ctx.enter_context(tc.tile_pool(name="wsmall", bufs=1)))
        apool = _patch_tile_name(ctx.enter_context(tc.tile_pool(name="acts", bufs=1)))
        kvpool = _patch_tile_name(ctx.enter_context(tc.tile_pool(name="kv", bufs=1)))
        epool = _patch_tile_name(ctx.enter_context(tc.tile_pool(name="eexp", bufs=1)))
        spool = _patch_tile_name(ctx.enter_context(tc.tile_pool(name="small", bufs=2)))

        ps_s = _patch_tile_name(ctx.enter_context(tc.tile_pool(name="ps_s", bufs=2, space="PSUM")))
        ps_g = _patch_tile_name(ctx.enter_context(tc.tile_pool(name="ps_g", bufs=4, space="PSUM")))

        # persistent weight tiles
        wq_t = [wqpool.tile([128, 3 * D], bf16, tag=f"wq{kk}") for kk in range(FT)]
        wp_t = [wpool.tile([128, D], bf16, tag=f"wp{kk}") for kk in range(FT)]
        w1_t = [wpool.tile([128, D], bf16, tag=f"w1{kk}") for kk in range(FT)]
        w2_t = [wpool.tile([128, D], bf16, tag=f"w2{kk}") for kk in range(FT)]

        # LN outputs (feature-major, both halves side by side)
        xhT = [ln_pool.tile([128, CHUNK], bf16, tag=f"xhT{t}") for t in range(FT)]
        hT = [ln_pool.tile([128, CHUNK], bf16, tag=f"hT{t}") for t in range(FT)]
        qT = [apool.tile([128, CHUNK], bf16, tag=f"qT{t}") for t in range(FT)]
        attnT = [apool.tile([128, CHUNK], bf16, tag=f"aT{t}") for t in range(FT)]
        gT = [apool.tile([128, CHUNK], bf16, tag=f"gT{t}") for t in range(FT)]
        kship = [apool.tile([128, 128], bf16, tag=f"ks{t}") for t in range(FT)]
        vship = apool.tile([128, D], bf16, tag="vs")
        # gathered K^T by kv parity: kall_p[par][ft] [128, 512] (4 chunks)
        kall_p = [[kvpool.tile([128, 4 * 128], bf16, tag=f"ka{par}_{t}")
                   for t in range(FT)] for par in range(2)]
        vext = [kvpool.tile([128, 16 * 65], bf16, tag=f"vext{t}") for t in range(KT)]

        def load_wq(li, what):
            for kk in range(FT):
                r0, r1 = 128 * kk, 128 * (kk + 1)
                if what == 'k':
                    nc.sync.dma_start(out=wq_t[kk][:, D:2 * D], in_=wq_in[li, r0:r1, D:2 * D])
                else:
                    nc.sync.dma_start(out=wq_t[kk][:, 0:D], in_=wq_in[li, r0:r1, 0:D])
                    nc.sync.dma_start(out=wq_t[kk][:, 2 * D:3 * D], in_=wq_in[li, r0:r1, 2 * D:3 * D])

        def load_w(dst, src, li):
            for kk in range(FT):
                nc.sync.dma_start(out=dst[kk], in_=src[li, 128 * kk:128 * (kk + 1), :])

        _ln_ctr = [0]

        def ln_stats(m):
            """Vector part of LN on x_t[m]: returns normalized bf16 tile."""
            stats = spool.tile([128, 2, 6], f32, tag="lnstats")
            nc.vector.bn_stats(out=stats[:, 0, :], in_=x_t[m][:, 0:512])
            nc.vector.bn_stats(out=stats[:, 1, :], in_=x_t[m][:, 512:1024])
            mv = spool.tile([128, 2], f32, tag="lnmv")
            nc.vector.bn_aggr(out=mv, in_=stats)
            # rstd = exp(-0.5*ln(var+eps)) — Ln and Exp share one ACT table
            # with attention's Exp, so the scalar engine never reloads tables.
            lv = spool.tile([128, 1], f32, tag="lnlv")
            nc.scalar.activation(out=lv, in_=mv[:, 1:2], func=AF.Ln, bias=eps_t)
            rs = spool.tile([128, 1], f32, tag="lnrs")
            nc.scalar.activation(out=rs, in_=lv, func=AF.Exp, scale=-0.5)
            xh_m = ln_pool.tile([128, D], bf16, tag=f"lnxh{_ln_ctr[0] % 2}")
            _ln_ctr[0] += 1
            nc.vector.tensor_scalar(
                out=xh_m, in0=x_t[m], scalar1=mv[:, 0:1], scalar2=rs,
                op0=mybir.AluOpType.subtract, op1=mybir.AluOpType.mult)
            return xh_m

        def ln_transposes(xh_m, m, dst):
            """PE part of LN: transpose xh_m into dst[t][:, 128m:128m+128]."""
            for t in range(FT):
                ptr = ps_g.tile([128, 128], bf16, tag="g", padded_shape=[128, 512])
                nc.tensor.transpose(ptr, xh_m[:, 128 * t:128 * (t + 1)], ident)
                eng = nc.vector if t % 2 == 0 else nc.scalar
                if eng is nc.scalar:
                    nc.scalar.copy(out=dst[t][:, 128 * m:128 * (m + 1)], in_=ptr)
                else:
                    nc.vector.tensor_copy(out=dst[t][:, 128 * m:128 * (m + 1)], in_=ptr)

        def layernorm_half(m, dst):
            ln_transposes(ln_stats(m), m, dst)

        def kv_half(l, m):
            """K^T,V for half m of layer l from xhT; ship; AllGather."""
            kv_loc, kv_all = kv_locs[l][m], kv_alls[l][m]
            c0 = 128 * m
            for ft in range(FT):
                ps = ps_g.tile([128, 512], f32, tag="g")
                for kk in range(FT):
                    nc.tensor.matmul(ps[:, 0:128], wq_t[kk][:, D + 128 * ft:D + 128 * (ft + 1)],
                                     xhT[kk][:, c0:c0 + 128],
                                     start=(kk == 0), stop=(kk == FT - 1))
                eng = nc.vector if ft % 2 == 0 else nc.scalar
                if eng is nc.scalar:
                    nc.scalar.copy(out=kship[ft], in_=ps[:, 0:128])
                else:
                    nc.vector.tensor_copy(out=kship[ft], in_=ps[:, 0:128])
                nc.sync.dma_start(
                    out=dram_ap(kv_loc, 128 * ft * HALF, [[HALF, 128], [1, HALF]]),
                    in_=kship[ft])
            for n in range(2):
                ps = ps_g.tile([128, 512], f32, tag="g")
                for kk in range(FT):
                    nc.tensor.matmul(
                        ps, xhT[kk][:, c0:c0 + 128],
                        wq_t[kk][:, 2 * D + 512 * n:2 * D + 512 * (n + 1)],
                        start=(kk == 0), stop=(kk == FT - 1))
                nc.vector.tensor_copy(out=vship[:, 512 * n:512 * (n + 1)], in_=ps)
            nc.sync.dma_start(
                out=dram_ap(kv_loc, D * HALF, [[D, 128], [1, D]]),
                in_=vship)
            nc.gpsimd.collective_compute(
                "AllGather", mybir.AluOpType.bypass, replica_groups=groups_b,
                ins=[kv_loc[:, :]], outs=[kv_all[:, :]])
            # pull this parity's gathered K/V as soon as the AG lands
            par = m
            for ft in range(FT):
                nc.sync.dma_start(
                    out=kall_p[par][ft].rearrange("p (r c) -> p r c", r=4),
                    in_=dram_ap(kv_all, 128 * ft * HALF,
                                [[HALF, 128], [KVH, 4], [1, HALF]]))
            for r in range(4):
                t = 2 * r + par
                ve = vext[t].rearrange("p (h c) -> p h c", h=16)
                nc.sync.dma_start(
                    out=ve[:, :, 0:64],
                    in_=dram_ap(kv_all, KVH * r + D * HALF,
                                [[D, 128], [64, 16], [1, 64]]))

        def q_both(l):
            for ft in range(FT):
                ps = ps_g.tile([128, 512], f32, tag="g")
                for kk in range(FT):
                    nc.tensor.matmul(ps[:, 0:CHUNK], wq_t[kk][:, 128 * ft:128 * (ft + 1)],
                                     xhT[kk], start=(kk == 0), stop=(kk == FT - 1))
                eng = nc.vector if ft % 2 == 0 else nc.scalar
                if eng is nc.scalar:
                    nc.scalar.copy(out=qT[ft], in_=ps[:, 0:CHUNK])
                else:
                    eng.tensor_copy(out=qT[ft], in_=ps[:, 0:CHUNK])

        def attention(l):
            def emit_scores(h, par):
                ft, ro = h // 2, 64 * (h % 2)
                s_ps = ps_s.tile([128, 4 * CHUNK], f32, tag="squad")
                for r in range(4):
                    nc.tensor.matmul(
                        s_ps[:, 256 * r:256 * (r + 1)],
                        kall_p[par][ft][ro:ro + 64, 128 * r:128 * (r + 1)],
                        qT[ft][ro:ro + 64, :], start=True, stop=True)
                e_q = epool.tile([128, 4 * CHUNK], bf16, tag="eq", bufs=2)
                nc.scalar.activation(out=e_q, in_=s_ps, func=AF.Exp, scale=0.125)
                em_q = epool.tile([128, 4 * CHUNK], bf16, tag=f"emq{par}",
                                  bufs=(4 if par == 0 else 2))
                nc.vector.tensor_mul(out=em_q, in0=e_q, in1=mask_t[par])
                return em_q

            def emit_av(h, em_pair):
                att_ps = ps_g.tile([65, CHUNK], f32, tag="g")
                for t in range(KT):
                    par, r = t % 2, t // 2
                    nc.tensor.matmul(att_ps, vext[t][:, 65 * h:65 * h + 65],
                                     em_pair[par][:, 256 * r:256 * (r + 1)],
                                     start=(t == 0), stop=(t == KT - 1))
                return att_ps

            def emit_epilogue(h, att_ps):
                ft, ro = h // 2, 64 * (h % 2)
                d_sb = spool.tile([1, CHUNK], f32, tag="denom", bufs=3)
                nc.vector.tensor_copy(out=d_sb, in_=att_ps[64:65, :])
                r_sb = spool.tile([1, CHUNK], f32, tag="recip", bufs=3)
                nc.vector.reciprocal_approx_fast(out=r_sb, in_=d_sb)
                rb_sb = spool.tile([64, CHUNK], f32, tag="rbsb", bufs=3)
                nc.gpsimd.partition_broadcast(rb_sb, r_sb, channels=64)
                nc.vector.tensor_mul(out=attnT[ft][ro:ro + 64, :],
                                     in0=att_ps[0:64, :], in1=rb_sb)

            DQ = 2   # odd-parity scores delayed DQ heads so AG_B has cover
            em_e, em_o, av_ps = {}, {}, {}
            for step in range(H + DQ + 2):
                if step < H:
                    em_e[step] = emit_scores(step, 0)
                ho = step - DQ
                if 0 <= ho < H:
                    em_o[ho] = emit_scores(ho, 1)
                ha = step - DQ - 1
                if 0 <= ha < H:
                    av_ps[ha] = emit_av(ha, [em_e.pop(ha), em_o.pop(ha)])
                he = step - DQ - 2
                if 0 <= he < H:
                    emit_epilogue(he, av_ps.pop(he))

        def proj_half(m):
            for n in range(2):
                ps = ps_g.tile([128, 512], f32, tag="g")
                for kk in range(FT):
                    nc.tensor.matmul(
                        ps, attnT[kk][:, 128 * m:128 * (m + 1)],
                        wp_t[kk][:, 512 * n:512 * (n + 1)],
                        start=(kk == 0), stop=(kk == FT - 1))
                nc.vector.tensor_add(
                    out=x_t[m][:, 512 * n:512 * (n + 1)],
                    in0=x_t[m][:, 512 * n:512 * (n + 1)], in1=ps)

        def fc1_half(m):
            c0 = 128 * m
            for f in range(FT):
                ps = ps_g.tile([128, 512], f32, tag="g")
                for kk in range(FT):
                    nc.tensor.matmul(ps[:, 0:128], w1_t[kk][:, 128 * f:128 * (f + 1)],
                                     hT[kk][:, c0:c0 + 128],
                                     start=(kk == 0), stop=(kk == FT - 1))
                nc.scalar.activation(out=gT[f][:, c0:c0 + 128], in_=ps[:, 0:128],
                                     func=AF.Gelu)

        def fc2_half(m):
            c0 = 128 * m
            for n in range(2):
                ps = ps_g.tile([128, 512], f32, tag="g")
                for kk in range(FT):
                    nc.tensor.matmul(
                        ps, gT[kk][:, c0:c0 + 128],
                        w2_t[kk][:, 512 * n:512 * (n + 1)],
                        start=(kk == 0), stop=(kk == FT - 1))
                nc.vector.tensor_add(
                    out=x_t[m][:, 512 * n:512 * (n + 1)],
                    in0=x_t[m][:, 512 * n:512 * (n + 1)], in1=ps)

        def lnf_ship(m, xfT):
            for t in range(FT):
                nc.sync.dma_start(
                    out=dram_ap(xf_locs[m], 128 * t * HALF, [[HALF, 128], [1, HALF]]),
                    in_=xfT[t][:, 128 * m:128 * (m + 1)])
            nc.gpsimd.collective_compute(
                "AllGather", mybir.AluOpType.bypass, replica_groups=group_all,
                ins=[xf_locs[m][:, :]], outs=[xf_alls[m][:, :]])

        def forward():
            # prologue: layer 0 weights, LN1 both halves, KV both, Q
            load_wq(0, 'k')
            load_wq(0, 'qv')
            load_w(wp_t, wp_in, 0)
            load_w(w1_t, w1_in, 0)
            load_w(w2_t, w2_in, 0)
            for t in range(KT):   # ones column for AV denominators, written once
                ve = vext[t].rearrange("p (h c) -> p h c", h=16)
                nc.gpsimd.memset(ve[:, :, 64:65], 1.0)
            layernorm_half(0, xhT)
            kv_half(0, 0)
            layernorm_half(1, xhT)
            kv_half(0, 1)
            q_both(0)

            xfT = hT  # reuse LN2 tiles for final LN output
            for l in range(n_layers):
                last = l + 1 >= n_layers
                attention(l)
                if not last:
                    load_wq(l + 1, 'k')
                    load_wq(l + 1, 'qv')
                proj_half(0)
                xh2a = ln_stats(0)          # LN2_A vector chain under proj_B
                proj_half(1)
                ln_transposes(xh2a, 0, hT)
                xh2b = ln_stats(1)          # LN2_B under LN2_A transposes
                ln_transposes(xh2b, 1, hT)
                fc1_half(0)                 # gelus of both halves contiguous
                fc1_half(1)                 # (one ACT table switch each way)
                fc2_half(0)                 # -> x_A of next layer
                xh1a = ln_stats(0)          # LN1_A(l+1) / lnf_A under fc2_B
                fc2_half(1)                 # -> x_B
                if not last:
                    ln_transposes(xh1a, 0, xhT)
                    kv_half(l + 1, 0)       # + AG_A + even-parity loads
                    xh1b = ln_stats(1)      # LN1_B under KV_A
                    load_w(wp_t, wp_in, l + 1)
                    load_w(w1_t, w1_in, l + 1)
                    load_w(w2_t, w2_in, l + 1)
                    ln_transposes(xh1b, 1, xhT)
                    kv_half(l + 1, 1)       # + AG_B + odd-parity loads
                    q_both(l + 1)
                else:
                    ln_transposes(xh1a, 0, xfT)
                    lnf_ship(0, xfT)
                    xh1b = ln_stats(1)
                    ln_transposes(xh1b, 1, xfT)
                    lnf_ship(1, xfT)

            # ---- logits: xall cols = [half0 ranks 0..7 | half1 ranks 0..7] ----
            xall = [wqpool.tile([128, N_CORES * HALF * 2], bf16, tag=f"wq{t}")
                    for t in range(FT)]
            for m in range(2):
                for t in range(FT):
                    nc.sync.dma_start(
                        out=xall[t].rearrange("p (r c) -> p r c", r=16)[:, 8 * m:8 * (m + 1), :],
                        in_=dram_ap(xf_alls[m], 128 * t * HALF,
                                    [[HALF, 128], [D * HALF, N_CORES], [1, HALF]]))
            NCH = [512] * 7 + [VS - 512 * 7]
            for n in range(8):
                n0 = 512 * n
                # w_out chunks double-buffer in the (now dead) wp/w1 storage
                wbank = wp_t if n % 2 == 0 else w1_t
                won = [wbank[kk][:, 0:NCH[n]] for kk in range(FT)]
                for kk in range(FT):
                    nc.sync.dma_start(out=won[kk],
                                      in_=wo_in[128 * kk:128 * (kk + 1), n0:n0 + NCH[n]])
                for mm in range(16):
                    # mm -> (half, rank): halves in order so AG_f1 overlaps
                    m, r = mm // 8, mm % 8
                    # core r holds batch r//4, chunk r%4; half m = tokens
                    # 256*(r%4)+128*m .. +128 of that batch
                    row0 = 1024 * (r // 4) + 256 * (r % 4) + 128 * m
                    ps = ps_g.tile([128, 512], f32, tag="g")
                    for kk in range(FT):
                        nc.tensor.matmul(
                            ps[:, :NCH[n]], xall[kk][:, 128 * mm:128 * (mm + 1)],
                            won[kk],
                            start=(kk == 0), stop=(kk == FT - 1))
                    lg = ln_pool.tile([128, 512], f32, tag="lg", bufs=2)
                    eng = nc.vector if mm % 2 == 0 else nc.scalar
                    if eng is nc.scalar:
                        nc.scalar.copy(out=lg[:, :NCH[n]], in_=ps[:, :NCH[n]])
                    else:
                        nc.vector.tensor_copy(out=lg[:, :NCH[n]], in_=ps[:, :NCH[n]])
                    nc.sync.dma_start(
                        out=out_ext[row0:row0 + 128, n0:n0 + NCH[n]],
                        in_=lg[:, :NCH[n]])

        forward()

    nc.compile()
    return nc


_CACHE = {}


def _get_program(n_layers=L, bcast_mode='mm'):
    key = (n_layers, bcast_mode)
    if key not in _CACHE:
        _CACHE[key] = build_program(n_layers, bcast_mode)
    return _CACHE[key]


def build_in_maps(prep, n_layers=L):
    in_maps = []
    for c in range(N_CORES):
        b, j = c // 4, c % 4
        in_maps.append({
            'x0': np.ascontiguousarray(prep['x0'][b, 256 * j:256 * (j + 1), :]),
            'wq': prep['w_qkv'][:n_layers],
            'wp': prep['w_proj'][:n_layers],
            'w1': prep['w_fc1'][:n_layers],
            'w2': prep['w_fc2'][:n_layers],
            'wo': np.ascontiguousarray(prep['w_out'][:, VS * c:VS * (c + 1)]),
            'masks': make_masks(j),
        })
    return in_maps


def unshard(res, n_rep=1):
    parts = [res.results[c]['logits'] for c in range(N_CORES)]   # [2048, 4000] each
    full = np.concatenate(parts, axis=1)                          # [2048, 32000]
    return full.reshape(B, T, V)


def run_model(prep, n_layers=L, bcast_mode='mm', **run_kwargs):
    from concourse.bass_utils import run_bass_kernel_spmd
    nc = _get_program(n_layers, bcast_mode)
    in_maps = build_in_maps(prep, n_layers)
    res = run_bass_kernel_spmd(nc, in_maps, core_ids=list(range(N_CORES)), **run_kwargs)
    return unshard(res)


def kernel(**inputs):
    prep = host_prep(inputs)
    return run_model(prep)
